# revision 25
# baseline (speedup 1.0000x reference)
"""LocalMHA (windowed attention, window=128, look_backward=1, RoPE) on 8 TRN2 cores.

Sharding: sequence-parallel, no collectives. Core c handles batch c//2,
sequence half c%2 (2048 query tokens + a 128-token look-backward halo whose
x rows ride along in the core's input shard; zeros at a true sequence start,
where the mask kills the backward keys anyway).

v2: fully fused single pass — no DRAM staging roundtrips (v1 spent ~60% of
its DMA on qrope/k2/vstage bounce buffers). Everything is bf16 (measured
gate error ~4.5e-3 against the 2e-2 budget):
  - x^T lands in SBUF via the DMA-engine transpose (InstDmaTransposeAnt,
    14ns/16x128 tile), eliminating v1's PE transposes + PSUM evictions.
  - QKV / out projections in bf16 (full PE rate at any free size).
  - q/k PSUM is evicted to SBUF bf16 (ACT), then RoPE runs on DVE at the
    2x 16-bit rate over [128, 8, nw, 128] views. Contiguous per-head
    layout: rotate partner of row r is r^32, so the sin product is 4
    quarter-ops (6 DVE ops per call); sin sign is folded into the tables.
  - The banded causal mask is ADDED BY THE PE: one matmul per pair with an
    identity stationary and the [mask|mask] tile as moving operand
    accumulates mask[i,j] into the scores PSUM — no DVE mask traffic.
  - Softmax per head-pair: 2 ACT exps [128,256] PSUM->SBUF bf16 with
    fused row-sum accumulators, then DVE reciprocal + two 4x-rate
    tensor_scalar normalize-and-store ops.
  - attn@v wants probs k-major: 4 PE transposes + one DVE copy per pair;
    the out-projection is fused per window (no full aT buffer).

Head-pair tasks are software-pipelined (stagger 3) so PE stays fed; chunk
c's QKV overlaps chunk c-1's attention, and x^T DMA-transposes are
prefetched one chunk ahead. GPSIMD/Pool is left idle on purpose — it
shares its SBUF port with DVE under an exclusive lock.
"""

import numpy as np
from contextlib import ExitStack
from ml_dtypes import bfloat16

import concourse.bacc as bacc
import concourse.tile as tile
import concourse.mybir as mybir
from concourse.bass_utils import run_bass_kernel_spmd
from concourse.masks import make_identity

# Problem shape (hardcoded per contract)
B, N, D = 4, 4096, 1024
H, DH, WS = 16, 64, 128
THETA = 10000.0
N3 = 3 * H * DH            # 3072
NCORES = 8
HALF = N // 2              # 2048 query tokens per core
NT = HALF + WS             # 2176 tokens incl halo window
SCALE = DH ** -0.5
NEG = -1.0e9
CW = 4                     # token-windows per chunk
NCH = 5                    # chunks (last has 1 window)

F32 = mybir.dt.float32
BF16 = mybir.dt.bfloat16
ADD = mybir.AluOpType.add
MUL = mybir.AluOpType.mult
EXP = mybir.ActivationFunctionType.Exp


def _build(reps=1):
    nc = bacc.Bacc("TRN2", target_bir_lowering=False, debug=False,
                   enable_asserts=False, num_devices=NCORES)

    xs = nc.dram_tensor("xs", [NT, D], BF16, kind="ExternalInput").ap()
    wq = nc.dram_tensor("wq", [D, N3], BF16, kind="ExternalInput").ap()
    wo = nc.dram_tensor("wo", [D, D], BF16, kind="ExternalInput").ap()
    # 6 tables x [128 rows, 8 nch * CW win * 128 cols] (tiled repeats)
    # 0:qcos 1:qsin 2:kcos_cur 3:ksin_cur 4:kcos_prev 5:ksin_prev
    ropes = nc.dram_tensor("ropes", [6, 128, 128], BF16,
                           kind="ExternalInput").ap()
    masks = nc.dram_tensor("masks", [2, 128, 512], BF16, kind="ExternalInput").ap()
    out = nc.dram_tensor("out", [HALF, D], F32, kind="ExternalOutput").ap()

    with tile.TileContext(nc) as tc:
        with ExitStack() as top:
            constp = top.enter_context(tc.tile_pool(name="const", bufs=1))
            identf = constp.tile([128, 128], F32, tag="idf")
            make_identity(nc, identf[:])
            identb = constp.tile([128, 128], BF16, tag="idb")
            nc.vector.tensor_copy(identb[:], identf[:])
            rp = constp.tile([128, 6, 1, 1, 128], BF16, tag="ropes")
            nc.sync.dma_start(rp[:, :, 0, 0, :], ropes.rearrange("r p m -> p r m"))
            mk = constp.tile([128, 2, 512], BF16, tag="masks")
            nc.sync.dma_start(mk[:], masks.rearrange("r p m -> p r m"))

            rep_ctx = tc.For_i(0, reps, 1) if reps > 1 else ExitStack()
            top.enter_context(rep_ctx)

            wp = top.enter_context(tc.tile_pool(name="wqp", bufs=1))
            w_sb = wp.tile([128, 8, N3], BF16, tag="w")
            nc.sync.dma_start(w_sb[:], wq.rearrange("(c p) n -> p c n", p=128))
            wop = top.enter_context(tc.tile_pool(name="wop", bufs=1))
            wo_sb = wop.tile([128, 8, D], BF16, tag="wo")
            nc.sync.dma_start(wo_sb[:], wo.rearrange("(c p) n -> p c n", p=128))

            xTp = top.enter_context(tc.tile_pool(name="xT", bufs=2))
            qrawp = top.enter_context(tc.tile_pool(name="qraw", bufs=1))
            krawp = top.enter_context(tc.tile_pool(name="kraw", bufs=1))
            qtp = top.enter_context(tc.tile_pool(name="qt", bufs=2))
            kkp = top.enter_context(tc.tile_pool(name="kk", bufs=2))
            vp = top.enter_context(tc.tile_pool(name="v", bufs=2))
            tmpp = top.enter_context(tc.tile_pool(name="tmp", bufs=1))
            eep = top.enter_context(tc.tile_pool(name="ee", bufs=4))
            pfp = top.enter_context(tc.tile_pool(name="pf", bufs=4))
            ptp = top.enter_context(tc.tile_pool(name="pt", bufs=4))
            sump = top.enter_context(tc.tile_pool(name="sums", bufs=6))
            aTp = top.enter_context(tc.tile_pool(name="aTw", bufs=3))
            osbp = top.enter_context(tc.tile_pool(name="osb", bufs=2))

            mps = top.enter_context(tc.tile_pool(name="mps", bufs=2, space="PSUM"))
            sps = top.enter_context(tc.tile_pool(name="sps", bufs=3, space="PSUM"))
            ptqp = top.enter_context(tc.tile_pool(name="ptq", bufs=2, space="PSUM"))
            avp_ = top.enter_context(tc.tile_pool(name="avp", bufs=1, space="PSUM"))

            # cross-chunk state (python refs; pool bufs sized to live ranges)
            kk_tiles = {}
            v_tiles = {}
            qt_tiles = {}
            xT_tiles = {}

            def nwof(c):
                return CW if c < NCH - 1 else 1

            def prefetch_xT(c):
                nw = nwof(c)
                L = 128 * nw
                t0 = CW * c
                xT = xTp.tile([128, 8, CW * 128], BF16, tag="xT", name="xT")
                nc.sync.dma_start_transpose(xT[:, :, 0:L],
                                            xs[t0 * 128: t0 * 128 + L, :])
                xT_tiles[c] = xT

            def tabv(i, nwv, r0, r1):
                # stride-0 broadcast over (nch, window): table is one window
                return rp[r0:r1, i].broadcast_to([r1 - r0, 8, nwv, 128])

            def rope(dst_f, src_f, ci, si, nwv):
                """dst = src*cos + rot32(src)*sin_signed (6 DVE ops, bf16).

                dst_f/src_f(r0, r1) -> [r1-r0, 8, nwv, 128] APs. Contiguous
                per-head layout: rotate partner of row r is r^32 within each
                64-row head block, so the sin product needs 4 quarter-ops
                (only the OUTPUT of an op may be partition-shifted; the sin
                tile is indexed by SOURCE row, destination sign folded in
                host-side).
                """
                t1 = tmpp.tile([128, 8, CW, 128], BF16, tag="t1")
                t2 = tmpp.tile([128, 8, CW, 128], BF16, tag="t2")
                nc.vector.tensor_tensor(t1[:, :, 0:nwv, :], src_f(0, 128),
                                        tabv(ci, nwv, 0, 128), MUL)
                for g in (0, 1):
                    lo, hi = g * 64, g * 64 + 32
                    nc.vector.tensor_tensor(t2[lo:lo + 32, :, 0:nwv, :],
                                            src_f(hi, hi + 32),
                                            tabv(si, nwv, hi, hi + 32), MUL)
                    nc.vector.tensor_tensor(t2[hi:hi + 32, :, 0:nwv, :],
                                            src_f(lo, lo + 32),
                                            tabv(si, nwv, lo, lo + 32), MUL)
                nc.vector.tensor_tensor(dst_f(0, 128), t1[:, :, 0:nwv, :],
                                        t2[:, :, 0:nwv, :], ADD)

            def emit_kproj(c):
                nw = nwof(c)
                L = 128 * nw
                t0 = CW * c
                xT = xT_tiles[c]

                if c + 1 <= NCH - 1 and (c + 1) not in kk_tiles:
                    kk_tiles[c + 1] = kkp.tile([128, 8, CW, 256], BF16,
                                               tag="kk", name="kk")
                if c not in kk_tiles:
                    kk_tiles[c] = kkp.tile([128, 8, CW, 256], BF16,
                                           tag="kk", name="kk")

                # K first: its ropes gate the next window group's scores, so
                # they run on DVE while PE chews the previous chunk's
                # attention matmuls.
                kraw = krawp.tile([128, 8, CW * 128], BF16, tag="kr")
                for nch in range(8):
                    mm = mps.tile([128, 512], F32, tag="mm")
                    for kc in range(8):
                        nc.tensor.matmul(
                            mm[:, 0:L],
                            w_sb[:, kc, 1024 + nch * 128: 1024 + (nch + 1) * 128],
                            xT[:, kc, 0:L],
                            start=(kc == 0), stop=(kc == 7))
                    nc.scalar.copy(kraw[:, nch, 0:L], mm[:, 0:L])

                # Per-window rope pieces, deferred: they interleave with the
                # previous chunk's attention pairs so no long serial rope
                # block ever stalls the fine-grained DVE stream.
                cs = 1 if c == 0 else 0     # halo window has no cur slot
                for mt in range(cs, nw):
                    def kcur_piece(mt=mt, kraw=kraw, c=c):
                        def kcdst(r0, r1):
                            return kk_tiles[c][r0:r1].rearrange(
                                "p c s (h m) -> p c s h m",
                                m=128)[:, :, mt:mt + 1, 1, :]

                        def kcsrc(r0, r1):
                            return kraw[r0:r1, :, :].rearrange(
                                "p c (w m) -> p c w m", m=128)[:, :, mt:mt + 1, :]

                        rope(kcdst, kcsrc, 2, 3, 1)
                    rope_pieces.append(kcur_piece)

                # kprv: token-window t feeds query window t+1's prv half
                for mt in range(nw):
                    t = t0 + mt
                    if t > 15:
                        continue
                    cw_, sw = (t + 1) // CW, (t + 1) % CW

                    def kprv_piece(mt=mt, kraw=kraw, cw_=cw_, sw=sw):
                        def kpdst(r0, r1):
                            return kk_tiles[cw_][r0:r1].rearrange(
                                "p c s (h m) -> p c s h m",
                                m=128)[:, :, sw:sw + 1, 0, :]

                        def kpsrc(r0, r1):
                            return kraw[r0:r1, :, :].rearrange(
                                "p c (w m) -> p c w m", m=128)[:, :, mt:mt + 1, :]

                        rope(kpdst, kpsrc, 4, 5, 1)
                    rope_pieces.append(kprv_piece)

            def emit_qproj(c):
                nw = nwof(c)
                L = 128 * nw
                xT = xT_tiles[c]
                qs = 128 if c == 0 else 0
                qraw = qrawp.tile([128, 8, CW * 128], BF16, tag="qr")
                for nch in range(8):
                    mm = mps.tile([128, 512], F32, tag="mm")
                    for kc in range(8):
                        nc.tensor.matmul(
                            mm[:, qs:L],
                            w_sb[:, kc, nch * 128:(nch + 1) * 128],
                            xT[:, kc, qs:L],
                            start=(kc == 0), stop=(kc == 7))
                    nc.scalar.copy(qraw[:, nch, qs:L], mm[:, qs:L])

                qt = qtp.tile([128, 8, CW * 128], BF16, tag="qt")
                qt_tiles[c] = qt
                w0q = qs // 128

                for mt in range(w0q, nw):
                    def q_piece(mt=mt, qraw=qraw, qt=qt):
                        def qdst(r0, r1):
                            return qt[r0:r1, :, :].rearrange(
                                "p c (w m) -> p c w m", m=128)[:, :, mt:mt + 1, :]

                        def qsrc(r0, r1):
                            return qraw[r0:r1, :, :].rearrange(
                                "p c (w m) -> p c w m", m=128)[:, :, mt:mt + 1, :]

                        rope(qdst, qsrc, 0, 1, 1)
                    rope_pieces.append(q_piece)

            def emit_vproj(c):
                nw = nwof(c)
                t0 = CW * c
                xT = xT_tiles.pop(c)
                # V natural [token, 1024] layout; evictions on DVE to keep
                # ACT free for the attention exps.
                vt = vp.tile([128, CW, D], BF16, tag="v")
                for mt in range(nw):
                    for nh in range(2):
                        vq = mps.tile([128, 512], F32, tag="mm")
                        for kc in range(8):
                            nc.tensor.matmul(
                                vq[:],
                                xT[:, kc, mt * 128:(mt + 1) * 128],
                                w_sb[:, kc, 2048 + nh * 512: 2048 + (nh + 1) * 512],
                                start=(kc == 0), stop=(kc == 7))
                        nc.scalar.copy(vt[:, mt, nh * 512:(nh + 1) * 512],
                                       vq[:])
                    v_tiles[t0 + mt] = (vt, mt)

            def emit_scores(w, blk):
                qt = qt_tiles[w // CW]
                kk = kk_tiles[w // CW]
                slot = w % CW
                sp = sps.tile([128, 512], F32, tag="s")
                mvar = 0 if w == 1 else 1
                for sub in range(2):
                    o = sub * 256
                    po = sub * 64
                    nc.tensor.matmul(
                        sp[:, o:o + 256],
                        qt[po:po + 64, blk, slot * 128:(slot + 1) * 128],
                        kk[po:po + 64, blk, slot, :],
                        start=True, stop=False)
                    nc.tensor.matmul(sp[:, o:o + 256], identb[:],
                                     mk[:, mvar, o:o + 256],
                                     start=False, stop=True)
                return sp

            def emit_rest(w, blk, sp, aTw):
                # Row sums alternate between ACT (fused exp accumulators) and
                # DVE (tensor_reduce) to balance the two engines.
                ee = eep.tile([128, 512], BF16, tag="ee")
                ss = sump.tile([128, 2], F32, tag="ss")
                if blk % 2 == 0:
                    for hh in range(2):
                        nc.scalar.activation(ee[:, hh * 256:(hh + 1) * 256],
                                             sp[:, hh * 256:(hh + 1) * 256], EXP,
                                             accum_out=ss[:, hh:hh + 1])
                else:
                    nc.scalar.activation(ee[:], sp[:], EXP)
                    nc.vector.tensor_reduce(
                        ss[:], ee[:].rearrange("p (h m) -> p h m", h=2),
                        axis=mybir.AxisListType.X, op=ADD)
                rr = sump.tile([128, 2], F32, tag="rr")
                nc.vector.reciprocal(rr[:], ss[:])
                pf = pfp.tile([128, 512], BF16, tag="pf")
                for hh in range(2):
                    nc.vector.tensor_scalar_mul(
                        pf[:, hh * 256:(hh + 1) * 256],
                        ee[:, hh * 256:(hh + 1) * 256], rr[:, hh:hh + 1])
                ptq = ptqp.tile([128, 512], BF16, tag="ptq")
                for j in range(4):
                    nc.tensor.transpose(ptq[:, j * 128:(j + 1) * 128],
                                        pf[:, j * 128:(j + 1) * 128], identb[:])
                pt = ptp.tile([128, 512], BF16, tag="pt")
                nc.vector.tensor_copy(pt[:], ptq[:])
                av = avp_.tile([128, 128], F32, tag="av")
                vprev, sprev = v_tiles[w - 1]
                vcur, scur = v_tiles[w]
                for sub in range(2):
                    d0 = blk * 128 + sub * 64
                    nc.tensor.matmul(av[sub * 64:(sub + 1) * 64, :],
                                     vprev[:, sprev, d0:d0 + 64],
                                     pt[:, sub * 256: sub * 256 + 128],
                                     start=True, stop=False)
                    nc.tensor.matmul(av[sub * 64:(sub + 1) * 64, :],
                                     vcur[:, scur, d0:d0 + 64],
                                     pt[:, sub * 256 + 128: sub * 256 + 256],
                                     start=False, stop=True)
                nc.scalar.copy(aTw[:, blk, :], av[:])

            def emit_outproj(w, aTw):
                osb = osbp.tile([128, D], F32, tag="o")
                for nh in range(2):
                    op_ = mps.tile([128, 512], F32, tag="mm")
                    for kc in range(8):
                        nc.tensor.matmul(op_[:], aTw[:, kc, :],
                                         wo_sb[:, kc, nh * 512:(nh + 1) * 512],
                                         start=(kc == 0), stop=(kc == 7))
                    nc.scalar.copy(osb[:, nh * 512:(nh + 1) * 512], op_[:])
                nc.sync.dma_start(out[(w - 1) * 128: w * 128, :], osb[:])

            # ---- software-pipelined main loop ----
            S = 2  # head-pair stagger depth
            pend = []
            aTw_tiles = {}
            rope_pieces = []

            drain_n = [0]

            def drain_one():
                w, blk, sp, aTw = pend.pop(0)
                emit_rest(w, blk, sp, aTw)
                drain_n[0] += 1
                if rope_pieces and drain_n[0] % 3 == 0:
                    rope_pieces.pop(0)()
                if blk == 7:
                    emit_outproj(w, aTw)
                    del aTw_tiles[w]

            def attn_windows(ws):
                for w in ws:
                    aTw_tiles[w] = aTp.tile([128, 8, 128], BF16, tag="aTw",
                                            name="aTw")
                    for blk in range(8):
                        sp = emit_scores(w, blk)
                        pend.append((w, blk, sp, aTw_tiles[w]))
                        while len(pend) > S:
                            drain_one()

            prefetch_xT(0)
            for c in range(NCH + 1):
                if c + 1 <= NCH - 1:
                    prefetch_xT(c + 1)
                if c < NCH:
                    emit_kproj(c)
                    emit_qproj(c)
                if c >= 1:
                    lo = CW * (c - 1)
                    ws = [t for t in range(lo, lo + CW) if 1 <= t <= 16]
                    attn_windows(ws)
                if c < NCH:
                    emit_vproj(c)
                while rope_pieces:
                    rope_pieces.pop(0)()
            while pend:
                drain_one()

    nc.compile()
    return nc


_NC = {}


def _get_nc(reps=1):
    if reps not in _NC:
        _NC[reps] = _build(reps)
    return _NC[reps]


# contiguous per-head layout: each 128-row block is [hA d0-63 | hB d0-63];
# rotate partner of row r is r^32 within each 64-row head block.
_r = np.arange(128)


def _host_inputs(x, W_qkv, W_out):
    Wb = np.ascontiguousarray(W_qkv, np.float32).astype(bfloat16)
    Wob = np.ascontiguousarray(W_out, np.float32).astype(bfloat16)

    invf = THETA ** (-(np.arange(0, 64, 2) / 64.0))          # [32]
    rows_f = invf[_r % 32]                                   # [128] freq per row
    # sin tiles are indexed by SOURCE row of the rotate (partner r^32);
    # the destination sign is +1 when the source is the lo half of its
    # 64-row head block (rot(t) = [-t_hi, t_lo]).
    rows_s = np.where((_r % 64) < 32, 1.0, -1.0)
    mcol = np.arange(128)
    angC = rows_f[:, None] * (128 + mcol)[None, :]
    angP = rows_f[:, None] * mcol[None, :]
    tabs = np.stack([
        SCALE * np.cos(angC),
        SCALE * (rows_s[:, None] * np.sin(angC)),
        np.cos(angC),
        rows_s[:, None] * np.sin(angC),
        np.cos(angP),
        rows_s[:, None] * np.sin(angP),
    ])                                                       # [6,128,128]
    ropes = tabs.astype(bfloat16)                            # [6,128,128]

    i = np.arange(128)[:, None]
    jj = np.arange(256)[None, :]
    band = (jj >= i) & (jj <= i + 128)
    maskB = np.where(band, 0.0, NEG).astype(np.float32)
    maskA0 = np.where(band & (jj >= 128), 0.0, NEG).astype(np.float32)
    mB2 = np.concatenate([maskB, maskB], axis=1).astype(bfloat16)
    mA2 = np.concatenate([maskA0, maskA0], axis=1).astype(bfloat16)

    in_maps = []
    for c in range(NCORES):
        bi, hi = c // 2, c % 2
        xsh = np.empty((NT, D), np.float32)
        if hi == 0:
            xsh[:WS] = 0.0
            xsh[WS:] = x[bi, 0:HALF]
            mA = mA2
        else:
            xsh[:] = x[bi, HALF - WS: N]
            mA = mB2
        in_maps.append({
            "xs": xsh.astype(bfloat16),
            "wq": Wb,
            "wo": Wob,
            "ropes": ropes,
            "masks": np.stack([mA, mB2]),
        })
    return in_maps


def kernel(x, W_qkv, W_out):
    x = np.asarray(x, np.float32)
    nc = _get_nc()
    in_maps = _host_inputs(x, W_qkv, W_out)
    res = run_bass_kernel_spmd(nc, in_maps, list(range(NCORES)))
    outf = np.empty((B, N, D), np.float32)
    for c in range(NCORES):
        bi, hi = c // 2, c % 2
        outf[bi, hi * HALF:(hi + 1) * HALF] = res.results[c]["out"]
    return outf


# revision 29
# speedup vs baseline: 1.1331x; 1.1331x over previous
"""LocalMHA (windowed attention, window=128, look_backward=1, RoPE) on 8 TRN2 cores.

Sharding: sequence-parallel, no collectives. Core c handles batch c//2,
sequence half c%2 (2048 query tokens + a 128-token look-backward halo whose
x rows ride along in the core's input shard; zeros at a true sequence start,
where the mask kills the backward keys anyway).

v2: fully fused single pass — no DRAM staging roundtrips (v1 spent ~60% of
its DMA on qrope/k2/vstage bounce buffers). Everything is bf16 (measured
gate error ~4.5e-3 against the 2e-2 budget):
  - x^T lands in SBUF via the DMA-engine transpose (InstDmaTransposeAnt,
    14ns/16x128 tile), eliminating v1's PE transposes + PSUM evictions.
  - QKV / out projections in bf16 (full PE rate at any free size).
  - q/k PSUM is evicted to SBUF bf16 (ACT), then RoPE runs on DVE at the
    2x 16-bit rate over [128, 8, nw, 128] views. Contiguous per-head
    layout: rotate partner of row r is r^32, so the sin product is 4
    quarter-ops (6 DVE ops per call); sin sign is folded into the tables.
  - The banded causal mask is ADDED BY THE PE: one matmul per pair with an
    identity stationary and the [mask|mask] tile as moving operand
    accumulates mask[i,j] into the scores PSUM — no DVE mask traffic.
  - Softmax per head-pair: 2 ACT exps [128,256] PSUM->SBUF bf16 with
    fused row-sum accumulators, then DVE reciprocal + two 4x-rate
    tensor_scalar normalize-and-store ops.
  - attn@v wants probs k-major: 4 PE transposes + one DVE copy per pair;
    the out-projection is fused per window (no full aT buffer).

Head-pair tasks are software-pipelined (stagger 3) so PE stays fed; chunk
c's QKV overlaps chunk c-1's attention, and x^T DMA-transposes are
prefetched one chunk ahead. GPSIMD/Pool is left idle on purpose — it
shares its SBUF port with DVE under an exclusive lock.
"""

import numpy as np
from contextlib import ExitStack
from ml_dtypes import bfloat16

import concourse.bacc as bacc
import concourse.tile as tile
import concourse.mybir as mybir
from concourse.bass_utils import run_bass_kernel_spmd
from concourse.masks import make_identity

# Problem shape (hardcoded per contract)
B, N, D = 4, 4096, 1024
H, DH, WS = 16, 64, 128
THETA = 10000.0
N3 = 3 * H * DH            # 3072
NCORES = 8
HALF = N // 2              # 2048 query tokens per core
NT = HALF + WS             # 2176 tokens incl halo window
SCALE = DH ** -0.5
NEG = -1.0e9
CW = 4                     # token-windows per chunk
NCH = 5                    # chunks (last has 1 window)

F32 = mybir.dt.float32
BF16 = mybir.dt.bfloat16
ADD = mybir.AluOpType.add
MUL = mybir.AluOpType.mult
EXP = mybir.ActivationFunctionType.Exp


def _build(reps=1):
    assert reps == 1 or reps % 2 == 0, "reps must be 1 or even"
    nc = bacc.Bacc("TRN2", target_bir_lowering=False, debug=False,
                   enable_asserts=False, num_devices=NCORES)

    xs = nc.dram_tensor("xs", [NT, D], BF16, kind="ExternalInput").ap()
    wq = nc.dram_tensor("wq", [D, N3], BF16, kind="ExternalInput").ap()
    wo = nc.dram_tensor("wo", [D, D], BF16, kind="ExternalInput").ap()
    # 6 tables x [128 rows, 8 nch * CW win * 128 cols] (tiled repeats)
    # 0:qcos 1:qsin 2:kcos_cur 3:ksin_cur 4:kcos_prev 5:ksin_prev
    ropes = nc.dram_tensor("ropes", [6, 128, 128], BF16,
                           kind="ExternalInput").ap()
    masks = nc.dram_tensor("masks", [2, 128, 512], BF16, kind="ExternalInput").ap()
    out = nc.dram_tensor("out", [HALF, D], F32, kind="ExternalOutput").ap()

    with tile.TileContext(nc) as tc:
        with ExitStack() as top:
            constp = top.enter_context(tc.tile_pool(name="const", bufs=1))
            identf = constp.tile([128, 128], F32, tag="idf")
            make_identity(nc, identf[:])
            identb = constp.tile([128, 128], BF16, tag="idb")
            nc.vector.tensor_copy(identb[:], identf[:])
            rp = constp.tile([128, 6, 1, 1, 128], BF16, tag="ropes")
            nc.sync.dma_start(rp[:, :, 0, 0, :], ropes.rearrange("r p m -> p r m"))
            mk = constp.tile([128, 2, 512], BF16, tag="masks")
            nc.sync.dma_start(mk[:], masks.rearrange("r p m -> p r m"))

            U = 2 if reps >= 2 else 1   # rep bodies per For_i iteration
            if reps > 1:
                top.enter_context(tc.For_i(0, reps // U, 1))

            wp = top.enter_context(tc.tile_pool(name="wqp", bufs=1))
            wop = top.enter_context(tc.tile_pool(name="wop", bufs=1))

            xTp = top.enter_context(tc.tile_pool(name="xT", bufs=2))
            qrawp = top.enter_context(tc.tile_pool(name="qraw", bufs=1))
            krawp = top.enter_context(tc.tile_pool(name="kraw", bufs=1))
            qtp = top.enter_context(tc.tile_pool(name="qt", bufs=2))
            kkp = top.enter_context(tc.tile_pool(name="kk", bufs=2))
            vp = top.enter_context(tc.tile_pool(name="v", bufs=2))
            tmpp = top.enter_context(tc.tile_pool(name="tmp", bufs=1))
            eep = top.enter_context(tc.tile_pool(name="ee", bufs=4))
            pfp = top.enter_context(tc.tile_pool(name="pf", bufs=4))
            ptp = top.enter_context(tc.tile_pool(name="pt", bufs=4))
            sump = top.enter_context(tc.tile_pool(name="sums", bufs=6))
            aTp = top.enter_context(tc.tile_pool(name="aTw", bufs=3))
            osbp = top.enter_context(tc.tile_pool(name="osb", bufs=2))

            mps = top.enter_context(tc.tile_pool(name="mps", bufs=2, space="PSUM"))
            sps = top.enter_context(tc.tile_pool(name="sps", bufs=4, space="PSUM"))
            ptqp = top.enter_context(tc.tile_pool(name="ptq", bufs=1, space="PSUM"))
            avp_ = top.enter_context(tc.tile_pool(name="avp", bufs=1, space="PSUM"))

            w_sb = None
            wo_sb = None
            # cross-chunk state (python refs; pool bufs sized to live ranges)
            kk_tiles = {}
            v_tiles = {}
            qt_tiles = {}
            xT_tiles = {}

            def nwof(c):
                return CW if c < NCH - 1 else 1

            def prefetch_xT(c):
                nw = nwof(c)
                L = 128 * nw
                t0 = CW * c
                xT = xTp.tile([128, 8, CW * 128], BF16, tag="xT", name="xT")
                nc.sync.dma_start_transpose(xT[:, :, 0:L],
                                            xs[t0 * 128: t0 * 128 + L, :])
                xT_tiles[c] = xT

            def tabv(i, nwv, r0, r1):
                # stride-0 broadcast over (nch, window): table is one window
                return rp[r0:r1, i].broadcast_to([r1 - r0, 8, nwv, 128])

            def rope(dst_f, src_f, ci, si, nwv):
                """dst = src*cos + rot32(src)*sin_signed (6 DVE ops, bf16).

                dst_f/src_f(r0, r1) -> [r1-r0, 8, nwv, 128] APs. Contiguous
                per-head layout: rotate partner of row r is r^32 within each
                64-row head block, so the sin product needs 4 quarter-ops
                (only the OUTPUT of an op may be partition-shifted; the sin
                tile is indexed by SOURCE row, destination sign folded in
                host-side).
                """
                t1 = tmpp.tile([128, 8, CW, 128], BF16, tag="t1")
                t2 = tmpp.tile([128, 8, CW, 128], BF16, tag="t2")
                nc.vector.tensor_tensor(t1[:, :, 0:nwv, :], src_f(0, 128),
                                        tabv(ci, nwv, 0, 128), MUL)
                for g in (0, 1):
                    lo, hi = g * 64, g * 64 + 32
                    nc.vector.tensor_tensor(t2[lo:lo + 32, :, 0:nwv, :],
                                            src_f(hi, hi + 32),
                                            tabv(si, nwv, hi, hi + 32), MUL)
                    nc.vector.tensor_tensor(t2[hi:hi + 32, :, 0:nwv, :],
                                            src_f(lo, lo + 32),
                                            tabv(si, nwv, lo, lo + 32), MUL)
                nc.vector.tensor_tensor(dst_f(0, 128), t1[:, :, 0:nwv, :],
                                        t2[:, :, 0:nwv, :], ADD)

            def emit_kproj(c):
                nw = nwof(c)
                L = 128 * nw
                t0 = CW * c
                xT = xT_tiles[c]

                if c + 1 <= NCH - 1 and (c + 1) not in kk_tiles:
                    kk_tiles[c + 1] = kkp.tile([128, 8, CW, 256], BF16,
                                               tag="kk", name="kk")
                if c not in kk_tiles:
                    kk_tiles[c] = kkp.tile([128, 8, CW, 256], BF16,
                                           tag="kk", name="kk")

                # K first: its ropes gate the next window group's scores, so
                # they run on DVE while PE chews the previous chunk's
                # attention matmuls.
                kraw = krawp.tile([128, 8, CW * 128], BF16, tag="kr")
                for nch in range(8):
                    mm = mps.tile([128, 512], F32, tag="mm")
                    for kc in range(8):
                        nc.tensor.matmul(
                            mm[:, 0:L],
                            w_sb[:, kc, 1024 + nch * 128: 1024 + (nch + 1) * 128],
                            xT[:, kc, 0:L],
                            start=(kc == 0), stop=(kc == 7))
                    nc.scalar.copy(kraw[:, nch, 0:L], mm[:, 0:L])

                # Per-window rope pieces, deferred: they interleave with the
                # previous chunk's attention pairs so no long serial rope
                # block ever stalls the fine-grained DVE stream.
                cs = 1 if c == 0 else 0     # halo window has no cur slot
                for mt in range(cs, nw):
                    def kcur_piece(mt=mt, kraw=kraw, c=c):
                        def kcdst(r0, r1):
                            return kk_tiles[c][r0:r1].rearrange(
                                "p c s (h m) -> p c s h m",
                                m=128)[:, :, mt:mt + 1, 1, :]

                        def kcsrc(r0, r1):
                            return kraw[r0:r1, :, :].rearrange(
                                "p c (w m) -> p c w m", m=128)[:, :, mt:mt + 1, :]

                        rope(kcdst, kcsrc, 2, 3, 1)
                    rope_pieces.append(kcur_piece)

                # kprv: token-window t feeds query window t+1's prv half
                for mt in range(nw):
                    t = t0 + mt
                    if t > 15:
                        continue
                    cw_, sw = (t + 1) // CW, (t + 1) % CW

                    def kprv_piece(mt=mt, kraw=kraw, cw_=cw_, sw=sw):
                        def kpdst(r0, r1):
                            return kk_tiles[cw_][r0:r1].rearrange(
                                "p c s (h m) -> p c s h m",
                                m=128)[:, :, sw:sw + 1, 0, :]

                        def kpsrc(r0, r1):
                            return kraw[r0:r1, :, :].rearrange(
                                "p c (w m) -> p c w m", m=128)[:, :, mt:mt + 1, :]

                        rope(kpdst, kpsrc, 4, 5, 1)
                    rope_pieces.append(kprv_piece)

            def emit_qproj(c):
                nw = nwof(c)
                L = 128 * nw
                xT = xT_tiles[c]
                qs = 128 if c == 0 else 0
                qraw = qrawp.tile([128, 8, CW * 128], BF16, tag="qr")
                for nch in range(8):
                    mm = mps.tile([128, 512], F32, tag="mm")
                    for kc in range(8):
                        nc.tensor.matmul(
                            mm[:, qs:L],
                            w_sb[:, kc, nch * 128:(nch + 1) * 128],
                            xT[:, kc, qs:L],
                            start=(kc == 0), stop=(kc == 7))
                    nc.scalar.copy(qraw[:, nch, qs:L], mm[:, qs:L])

                qt = qtp.tile([128, 8, CW * 128], BF16, tag="qt")
                qt_tiles[c] = qt
                w0q = qs // 128

                for mt in range(w0q, nw):
                    def q_piece(mt=mt, qraw=qraw, qt=qt):
                        def qdst(r0, r1):
                            return qt[r0:r1, :, :].rearrange(
                                "p c (w m) -> p c w m", m=128)[:, :, mt:mt + 1, :]

                        def qsrc(r0, r1):
                            return qraw[r0:r1, :, :].rearrange(
                                "p c (w m) -> p c w m", m=128)[:, :, mt:mt + 1, :]

                        rope(qdst, qsrc, 0, 1, 1)
                    rope_pieces.append(q_piece)

            def emit_vproj(c):
                nw = nwof(c)
                t0 = CW * c
                xT = xT_tiles.pop(c)
                # V natural [token, 1024] layout; evictions on DVE to keep
                # ACT free for the attention exps.
                vt = vp.tile([128, CW, D], BF16, tag="v")
                for mt in range(nw):
                    for nh in range(2):
                        vq = mps.tile([128, 512], F32, tag="mm")
                        for kc in range(8):
                            nc.tensor.matmul(
                                vq[:],
                                xT[:, kc, mt * 128:(mt + 1) * 128],
                                w_sb[:, kc, 2048 + nh * 512: 2048 + (nh + 1) * 512],
                                start=(kc == 0), stop=(kc == 7))
                        nc.scalar.copy(vt[:, mt, nh * 512:(nh + 1) * 512],
                                       vq[:])
                    v_tiles[t0 + mt] = (vt, mt)

            def emit_scores(w, blk):
                qt = qt_tiles[w // CW]
                kk = kk_tiles[w // CW]
                slot = w % CW
                sp = sps.tile([128, 512], F32, tag="s")
                mvar = 0 if w == 1 else 1
                for sub in range(2):
                    o = sub * 256
                    po = sub * 64
                    nc.tensor.matmul(
                        sp[:, o:o + 256],
                        qt[po:po + 64, blk, slot * 128:(slot + 1) * 128],
                        kk[po:po + 64, blk, slot, :],
                        start=True, stop=False)
                    nc.tensor.matmul(sp[:, o:o + 256], identb[:],
                                     mk[:, mvar, o:o + 256],
                                     start=False, stop=True)
                return sp

            def emit_rest(w, blk, sp, aTw):
                # Row sums alternate between ACT (fused exp accumulators) and
                # DVE (tensor_reduce) to balance the two engines.
                ee = eep.tile([128, 512], BF16, tag="ee")
                ss = sump.tile([128, 2], F32, tag="ss")
                if blk % 2 == 0:
                    for hh in range(2):
                        nc.scalar.activation(ee[:, hh * 256:(hh + 1) * 256],
                                             sp[:, hh * 256:(hh + 1) * 256], EXP,
                                             accum_out=ss[:, hh:hh + 1])
                else:
                    nc.scalar.activation(ee[:], sp[:], EXP)
                    nc.vector.tensor_reduce(
                        ss[:], ee[:].rearrange("p (h m) -> p h m", h=2),
                        axis=mybir.AxisListType.X, op=ADD)
                rr = sump.tile([128, 2], F32, tag="rr")
                nc.vector.reciprocal(rr[:], ss[:])
                pf = pfp.tile([128, 512], BF16, tag="pf")
                for hh in range(2):
                    nc.vector.tensor_scalar_mul(
                        pf[:, hh * 256:(hh + 1) * 256],
                        ee[:, hh * 256:(hh + 1) * 256], rr[:, hh:hh + 1])
                ptq = ptqp.tile([128, 512], BF16, tag="ptq")
                for j in range(4):
                    nc.tensor.transpose(ptq[:, j * 128:(j + 1) * 128],
                                        pf[:, j * 128:(j + 1) * 128], identb[:])
                pt = ptp.tile([128, 512], BF16, tag="pt")
                nc.vector.tensor_copy(pt[:], ptq[:])
                av = avp_.tile([128, 128], F32, tag="av")
                vprev, sprev = v_tiles[w - 1]
                vcur, scur = v_tiles[w]
                for sub in range(2):
                    d0 = blk * 128 + sub * 64
                    nc.tensor.matmul(av[sub * 64:(sub + 1) * 64, :],
                                     vprev[:, sprev, d0:d0 + 64],
                                     pt[:, sub * 256: sub * 256 + 128],
                                     start=True, stop=False)
                    nc.tensor.matmul(av[sub * 64:(sub + 1) * 64, :],
                                     vcur[:, scur, d0:d0 + 64],
                                     pt[:, sub * 256 + 128: sub * 256 + 256],
                                     start=False, stop=True)
                nc.scalar.copy(aTw[:, blk, :], av[:])

            def emit_outproj(w, aTw):
                osb = osbp.tile([128, D], F32, tag="o")
                for nh in range(2):
                    op_ = mps.tile([128, 512], F32, tag="mm")
                    for kc in range(8):
                        nc.tensor.matmul(op_[:], aTw[:, kc, :],
                                         wo_sb[:, kc, nh * 512:(nh + 1) * 512],
                                         start=(kc == 0), stop=(kc == 7))
                    nc.scalar.copy(osb[:, nh * 512:(nh + 1) * 512], op_[:])
                nc.sync.dma_start(out[(w - 1) * 128: w * 128, :], osb[:])

            # ---- software-pipelined main loop ----
            S = 3  # head-pair stagger depth
            pend = []
            aTw_tiles = {}
            rope_pieces = []

            drain_n = [0]

            def drain_one():
                w, blk, sp, aTw = pend.pop(0)
                emit_rest(w, blk, sp, aTw)
                drain_n[0] += 1
                if rope_pieces and drain_n[0] % 3 == 0:
                    rope_pieces.pop(0)()
                if blk == 7:
                    emit_outproj(w, aTw)
                    del aTw_tiles[w]

            def attn_windows(ws):
                for w in ws:
                    aTw_tiles[w] = aTp.tile([128, 8, 128], BF16, tag="aTw",
                                            name="aTw")
                    for blk in range(8):
                        sp = emit_scores(w, blk)
                        pend.append((w, blk, sp, aTw_tiles[w]))
                        while len(pend) > S:
                            drain_one()

            def emit_rep():
                nonlocal w_sb, wo_sb
                w_sb = wp.tile([128, 8, N3], BF16, tag="w", name="w_sb")
                nc.sync.dma_start(w_sb[:],
                                  wq.rearrange("(c p) n -> p c n", p=128))
                wo_sb = wop.tile([128, 8, D], BF16, tag="wo", name="wo_sb")
                nc.sync.dma_start(wo_sb[:],
                                  wo.rearrange("(c p) n -> p c n", p=128))
                kk_tiles.clear()
                v_tiles.clear()
                qt_tiles.clear()
                xT_tiles.clear()
                prefetch_xT(0)
                for c in range(NCH + 1):
                    if c + 1 <= NCH - 1:
                        prefetch_xT(c + 1)
                    if c < NCH:
                        emit_kproj(c)
                        emit_qproj(c)
                    if c >= 1:
                        lo = CW * (c - 1)
                        ws = [t for t in range(lo, lo + CW) if 1 <= t <= 16]
                        attn_windows(ws)
                    if c < NCH:
                        emit_vproj(c)
                    while rope_pieces:
                        rope_pieces.pop(0)()
                while pend:
                    drain_one()

            for _ in range(U if reps > 1 else 1):
                emit_rep()

    nc.compile()
    return nc


_NC = {}


def _get_nc(reps=1):
    if reps not in _NC:
        _NC[reps] = _build(reps)
    return _NC[reps]


# contiguous per-head layout: each 128-row block is [hA d0-63 | hB d0-63];
# rotate partner of row r is r^32 within each 64-row head block.
_r = np.arange(128)


def _host_inputs(x, W_qkv, W_out):
    Wb = np.ascontiguousarray(W_qkv, np.float32).astype(bfloat16)
    Wob = np.ascontiguousarray(W_out, np.float32).astype(bfloat16)

    invf = THETA ** (-(np.arange(0, 64, 2) / 64.0))          # [32]
    rows_f = invf[_r % 32]                                   # [128] freq per row
    # sin tiles are indexed by SOURCE row of the rotate (partner r^32);
    # the destination sign is +1 when the source is the lo half of its
    # 64-row head block (rot(t) = [-t_hi, t_lo]).
    rows_s = np.where((_r % 64) < 32, 1.0, -1.0)
    mcol = np.arange(128)
    angC = rows_f[:, None] * (128 + mcol)[None, :]
    angP = rows_f[:, None] * mcol[None, :]
    tabs = np.stack([
        SCALE * np.cos(angC),
        SCALE * (rows_s[:, None] * np.sin(angC)),
        np.cos(angC),
        rows_s[:, None] * np.sin(angC),
        np.cos(angP),
        rows_s[:, None] * np.sin(angP),
    ])                                                       # [6,128,128]
    ropes = tabs.astype(bfloat16)                            # [6,128,128]

    i = np.arange(128)[:, None]
    jj = np.arange(256)[None, :]
    band = (jj >= i) & (jj <= i + 128)
    maskB = np.where(band, 0.0, NEG).astype(np.float32)
    maskA0 = np.where(band & (jj >= 128), 0.0, NEG).astype(np.float32)
    mB2 = np.concatenate([maskB, maskB], axis=1).astype(bfloat16)
    mA2 = np.concatenate([maskA0, maskA0], axis=1).astype(bfloat16)

    in_maps = []
    for c in range(NCORES):
        bi, hi = c // 2, c % 2
        xsh = np.empty((NT, D), np.float32)
        if hi == 0:
            xsh[:WS] = 0.0
            xsh[WS:] = x[bi, 0:HALF]
            mA = mA2
        else:
            xsh[:] = x[bi, HALF - WS: N]
            mA = mB2
        in_maps.append({
            "xs": xsh.astype(bfloat16),
            "wq": Wb,
            "wo": Wob,
            "ropes": ropes,
            "masks": np.stack([mA, mB2]),
        })
    return in_maps


def kernel(x, W_qkv, W_out):
    x = np.asarray(x, np.float32)
    nc = _get_nc()
    in_maps = _host_inputs(x, W_qkv, W_out)
    res = run_bass_kernel_spmd(nc, in_maps, list(range(NCORES)))
    outf = np.empty((B, N, D), np.float32)
    for c in range(NCORES):
        bi, hi = c // 2, c % 2
        outf[bi, hi * HALF:(hi + 1) * HALF] = res.results[c]["out"]
    return outf
